# revision 6
# baseline (speedup 1.0000x reference)
"""Trainium2 Bass kernel: 3-layer actor MLP over [B=256, K=1000] actions.

Math (per reference):
    h1 = relu(af @ W1_a + state @ W1_s + b1)   # [B,K,256]
    h2 = relu(h1 @ W2 + b2)                    # [B,K,128]
    out = h2 @ W3 + b3                         # [B,K]

Sharding: data-parallel over B across 8 NeuronCores (32 rows each);
weights replicated.  Compute in bf16 (f32 PSUM accumulate).

v2 design notes (measured bottleneck: PSUM evacuation, not matmul).
PSUM can only be read by ScalarE (1.2 elem/ns/lane) and VectorE
(0.96 elem/ns/lane); per-step relu+bias evacuation of h1 (2048
lane-elems) + h2 (1024) is the pace-setter.  So:
  * Epilogues are merged 2:1 into single contiguous [128, 1012] ops
    (both 512-col k-chunks of one PSUM bank-pair in one instruction,
    skipping only the final 12 pad cols), amortizing the fixed op
    overhead (ScE 352 cyc, DVE 120 cyc).  3 eps/step instead of 6.
  * One shared PSUM pool (3 bufs x [128,1024] = 6 banks) serves L1's
    two h-half tiles and L2's merged tile, keeping a full step of
    ring slack; L3 holds the 7th bank.
  * L3 uses col-tiled matmuls: score row (b, ch) lands on PSUM
    partition 32*ch + b via a [128,32] stationary (w3 strip window)
    targeting col-group ch; the two per-step matmuls hit different
    col-groups so they stream CONCURRENTLY (~210ns/step vs 420).
    All 64 rows accumulate in one bank; one DVE copy + one DMA out.
  * L2 matmuls run hh-outer so each W2 half is loaded once per step.
TensorE per step ~1.5us < epilogue ~1.75us -> epilogue-bound cadence.

Device tensors per core (host pre-packs, all contiguous):
  a2     [4,128,4096] bf16  action_feats.T in contiguous 8-row groups
                            (per row: p 0:64 = k 0:500, p 64:128 =
                            k 500:1000; cols 500:512 zero pad) so each
                            input DMA is a single contiguous transfer
  consts [128,1727]   bf16  w1s(4x128 chunks)|state.T|w1a(dup)|w2|w3strip
  biases [128,3]      f32   b1 (2 cols) | b2  (b3 added host-side)
  out    [32,1000]    f32   final scores (host adds b3 only)
"""

import os
import numpy as np

B, K = 256, 1000
SD, AD, H, G = 512, 64, 256, 128
NCORES = 8
BL = B // NCORES          # 32 batch rows per core
KC = 500                  # real k-chunk length (2 chunks per row)
KP = 512                  # padded k-chunk length (fills one PSUM bank)
EPW = 2 * KP - (KP - KC)  # 1012: merged-epilogue width (skip last pad)
GRP = 8                   # batch rows per input-DMA group
NCONST = 1856             # bf16 cols: w1s|s2|w1a|w2|w3strip

_CACHE = {}
LAST_EXEC_NS = None


def _build_nc():
    from contextlib import ExitStack

    import concourse.bass as bass
    import concourse.bacc as bacc
    import concourse.mybir as mybir
    import concourse.tile as tile

    f32 = mybir.dt.float32
    bf16 = mybir.dt.bfloat16
    AF = mybir.ActivationFunctionType
    ALU = mybir.AluOpType

    nc = bacc.Bacc("TRN2", target_bir_lowering=False, debug=False,
                   num_devices=NCORES)

    a2 = nc.dram_tensor("a2", [BL // GRP, 128, GRP * KP], bf16,
                        kind="ExternalInput").ap()
    constsd = nc.dram_tensor("consts", [128, NCONST], bf16,
                             kind="ExternalInput").ap()
    biasd = nc.dram_tensor("biases", [128, 3], f32, kind="ExternalInput").ap()
    out = nc.dram_tensor("out", [BL, 2 * KC], f32,
                         kind="ExternalOutput").ap()

    with tile.TileContext(nc) as tc, ExitStack() as ctx:
        wp = ctx.enter_context(tc.tile_pool(name="wp", bufs=1))
        xp = ctx.enter_context(tc.tile_pool(name="xp", bufs=3))
        h1p = ctx.enter_context(tc.tile_pool(name="h1p", bufs=6))
        h2p = ctx.enter_context(tc.tile_pool(name="h2p", bufs=3))
        osp = ctx.enter_context(tc.tile_pool(name="osp", bufs=1))
        # shared PSUM pool: L1 h-half tiles + L2 merged tile (2 banks each)
        pp = ctx.enter_context(tc.tile_pool(name="pp", bufs=3, space="PSUM"))
        l3p = ctx.enter_context(tc.tile_pool(name="l3p", bufs=1, space="PSUM"))

        # ---- ACT table preload: fire Relu once so the 1.3us table load
        # overlaps the input DMAs instead of blocking the first epilogue ----
        da = wp.tile([128, 2], f32)
        nc.gpsimd.memset(da[:], 0.0)
        nc.scalar.activation(da[:, 1:2], da[:, 0:1], AF.Relu)

        # ---- constants: single DMA so matmuls wait on one semaphore ----
        cs = wp.tile([128, NCONST], bf16)
        nc.sync.dma_start(cs[:], constsd[:])
        w1s_sb = cs[:, 0:1024].rearrange("p (c h) -> p c h", c=4)
        s2_sb = cs[:, 1024:1152]
        w1a_sb = cs[:, 1152:1408]
        w2_sb = cs[:, 1408:1664].rearrange("p (c g) -> p c g", c=2)
        w3s_sb = cs[:, 1664:1856]

        # ---- input prefetch (xt0 early: L1(b0) is the critical path) ----
        xts = {}

        def stage_xt(g):
            if g < BL // GRP and g not in xts:
                xt = xp.tile([128, GRP, KP], bf16, tag="xt")
                xts[g] = xt
                nc.sync.dma_start(
                    xts[g][:].rearrange("p j y -> p (j y)"), a2[g, :, :])

        stage_xt(0)
        bb = wp.tile([128, 3], f32)
        nc.sync.dma_start(bb[:], biasd[:])
        b1_sb = bb[:, 0:2]
        b2_sb = bb[:, 2:3]
        stage_xt(1)

        # ---- epilogue engine balancer (ACT vs DVE), HW cost models ----
        eng_ns = [0.0, 0.0]

        def ep(out_ap, in_ap, bias_ap, relu, eng=None):
            fd = in_ap.free_size()
            cost_act = (fd + 352) / 1.2
            cost_dve = (fd + 120) / 0.96
            if eng is None:
                eng = 0 if eng_ns[0] + cost_act <= eng_ns[1] + cost_dve else 1
            with tc.high_priority():
                if eng == 0:
                    eng_ns[0] += cost_act
                    return nc.scalar.activation(
                        out_ap, in_ap, AF.Relu if relu else AF.Identity,
                        bias=bias_ap)
                eng_ns[1] += cost_dve
                if relu:
                    return nc.vector.tensor_scalar(out_ap, in_ap, bias_ap,
                                                   0.0, ALU.add, ALU.max)
                return nc.vector.tensor_scalar(out_ap, in_ap, bias_ap, None,
                                               ALU.add)

        # ---- PE warm-up while the first DMAs land (HAM K=8/8 window) ----
        dummy = wp.tile([64, 576], bf16)
        nc.vector.memset(dummy[:], 0.0)
        wps = pp.tile([128, 1024], f32, tag="pp")
        for _ in range(12):
            nc.tensor.matmul(wps[0:64, 0:512], lhsT=dummy[:, 512:576],
                             rhs=dummy[:, 0:512], start=True, stop=True)

        # ---- h_state = (state @ W1_s).T + b1 : [128, 64], col 32h+b ----
        # (transient pp PSUM + DVE-forced epilogue so neither blocks L1(b0))
        hs_sb = wp.tile([128, 2 * BL], f32)
        hsps = pp.tile([128, 1024], f32, tag="pp")
        for h in range(2):
            for c in range(4):
                nc.tensor.matmul(
                    hsps[:, 32 * h:32 * (h + 1)],
                    lhsT=w1s_sb[:, c, 128 * h:128 * (h + 1)],
                    rhs=s2_sb[:, 32 * c:32 * (c + 1)],
                    start=(c == 0), stop=(c == 3))
            with tc.high_priority():
                nc.vector.tensor_scalar(
                    hs_sb[:, 32 * h:32 * (h + 1)],
                    hsps[:, 32 * h:32 * (h + 1)],
                    b1_sb[:, h:h + 1], None, ALU.add)
                eng_ns[1] += 160

        # ---- per-step stages (software-pipelined: L1(s), L2(s-1),
        # L3(s-2)) ----

        def stage_l1h(b, h):
            # one [128,1024] PSUM bank-pair; the two k-chunk matmuls run
            # concurrently on row-groups 0:64 / 64:128, then ONE merged
            # relu+bias epilogue evacuates both banks in a single op.
            g, j = divmod(b, GRP)
            h1 = h1p.tile([128, 1024], bf16, tag="h1")
            l1t = pp.tile([128, 1024], f32, tag="pp")
            for c in range(2):
                nc.tensor.matmul(
                    l1t[:, KP * c:KP * c + KP],
                    lhsT=w1a_sb[64 * c:64 * (c + 1),
                                128 * h:128 * (h + 1)],
                    rhs=xts[g][64 * c:64 * (c + 1), j, :],
                    start=True, stop=True)
            ep(h1[:, 0:EPW], l1t[:, 0:EPW],
               hs_sb[:, 32 * h + b:32 * h + b + 1], relu=True)
            return h1

        def stage_l2(b, h1t):
            # merged [128,1024] PSUM tile; hh-outer so each W2 half is one
            # LDWEIGHTS; ONE merged relu+bias epilogue.
            h2 = h2p.tile([128, 1024], bf16, tag="h2")
            l2t = pp.tile([128, 1024], f32, tag="pp")
            for hh in range(2):
                for c in range(2):
                    nc.tensor.matmul(
                        l2t[:, KP * c:KP * c + KP],
                        lhsT=w2_sb[:, hh, :],
                        rhs=h1t[hh][:, KP * c:KP * c + KP],
                        start=(hh == 0), stop=(hh == 1))
            ep(h2[:, 0:EPW], l2t[:, 0:EPW], b2_sb[:, 0:1], relu=True)
            return h2

        # L3: score row (b, ch) lands on PSUM partition p = 2b+ch via a
        # full-array matmul whose stationary is zeros except col p = w3
        # (sliding window of w3s_sb); all 64 matmuls accumulate into one
        # bank (+0.0 on every other partition), one copy + one DMA out.
        l3state = {}

        def stage_l3(b, h2):
            if b == 0:
                l3state[0] = l3p.tile([128, 512], f32, tag="l3",
                                      name="l3bank")
            l3b = l3state[0]
            for ch in range(2):
                p = 2 * b + ch
                nc.tensor.matmul(
                    l3b[:, 0:KC],
                    lhsT=w3s_sb[:, 63 - p:191 - p],
                    rhs=h2[:, 0:KC] if ch == 0 else h2[:, KP:KP + KC],
                    start=(p == 0), stop=(p == 2 * BL - 1))
            if b == BL - 1:
                osb = osp.tile([2 * BL, 500], f32, tag="osb")
                with tc.high_priority():
                    nc.vector.tensor_scalar(osb[:], l3b[0:2 * BL, 0:KC], 0.0,
                                            None, ALU.add)
                    eng_ns[1] += 650
                del l3state[0]
                nc.sync.dma_start(
                    out[:].rearrange("b (c y) -> (b c) y", c=2), osb[:])

        pend = {}
        pend2 = {}
        for s in range(BL + 2):
            if s < BL:
                if s % GRP == 0:
                    stage_xt(s // GRP + 1)
                h1a = stage_l1h(s, 0)
                h1b = stage_l1h(s, 1)
            if s - 1 in pend:
                pend2[s - 1] = stage_l2(s - 1, pend.pop(s - 1))
            if s - 2 in pend2:
                stage_l3(s - 2, pend2.pop(s - 2))
            if s < BL:
                pend[s] = [h1a, h1b]

    nc.compile()
    return nc


def _prep_inputs(state_embed, action_feats, W1, b1, W2, b2, W3, b3):
    import ml_dtypes
    bf = ml_dtypes.bfloat16
    f4 = lambda x: np.ascontiguousarray(np.asarray(x, dtype=np.float32))
    state_embed, action_feats = f4(state_embed), f4(action_feats)
    W1, b1, W2, b2, W3, b3 = map(f4, (W1, b1, W2, b2, W3, b3))

    W1s, W1a = W1[:SD], W1[SD:]
    w1s_h = np.concatenate([W1s[c * 128:(c + 1) * 128] for c in range(4)],
                           axis=1).astype(bf)                # [128, 1024]
    w1a_h = np.concatenate([W1a, W1a], axis=0).astype(bf)    # [128, 256]
    w2_h = np.concatenate([W2[:128], W2[128:]], axis=1).astype(bf)
    w3s_h = np.zeros((G, 192), dtype=np.float32)
    w3s_h[:, 63] = W3[:, 0]
    w3s_h = w3s_h.astype(bf)
    biases = np.ascontiguousarray(np.concatenate(
        [b1.reshape(2, 128).T, b2.reshape(G, 1)], axis=1))   # [128, 3] f32

    in_maps = []
    for ci in range(NCORES):
        sl = slice(ci * BL, (ci + 1) * BL)
        aft = action_feats[sl].transpose(0, 2, 1)            # [BL, 64, 1000]
        a2_h = np.zeros((BL, 128, KP), dtype=bf)
        a2_h[:, 0:64, 0:KC] = aft[:, :, :KC].astype(bf)
        a2_h[:, 64:128, 0:KC] = aft[:, :, KC:].astype(bf)
        a2_h = np.ascontiguousarray(
            a2_h.reshape(BL // GRP, GRP, 128, KP).transpose(0, 2, 1, 3)
            .reshape(BL // GRP, 128, GRP * KP))
        st = state_embed[sl].T.astype(bf)                    # [512, BL]
        s2_h = np.concatenate([st[c * 128:(c + 1) * 128] for c in range(4)],
                              axis=1)                        # [128, 128]
        consts = np.ascontiguousarray(np.concatenate(
            [w1s_h, s2_h, w1a_h, w2_h, w3s_h], axis=1))
        assert consts.shape == (128, NCONST), consts.shape
        in_maps.append({"a2": a2_h, "consts": consts, "biases": biases})
    return in_maps, float(b3.reshape(-1)[0])


def kernel(state_embed, action_feats, W1, b1, W2, b2, W3, b3):
    global LAST_EXEC_NS
    from concourse.bass_utils import run_bass_kernel_spmd

    if "nc" not in _CACHE:
        _CACHE["nc"] = _build_nc()
    nc = _CACHE["nc"]

    in_maps, b3v = _prep_inputs(state_embed, action_feats, W1, b1, W2, b2,
                                W3, b3)
    trace = bool(int(os.environ.get("ACTOR_KERNEL_TRACE", "0")))
    res = run_bass_kernel_spmd(nc, in_maps, core_ids=list(range(NCORES)),
                               trace=trace)
    LAST_EXEC_NS = res.exec_time_ns
    outs = [np.asarray(res.results[i]["out"]) for i in range(NCORES)]
    return np.ascontiguousarray(
        (np.concatenate(outs, axis=0) + b3v).astype(np.float32))
